# revision 1
# baseline (speedup 1.0000x reference)
"""Trainium2 Bass kernel for a dense transformer decoder block on 8 NeuronCores.

Sharding (uniform SPMD):
  * tokens: core c owns 512 contiguous tokens — batch c//4, positions
    [512*(c%4), 512*(c%4)+512). All projections, norms and the FFN are
    computed purely locally on those tokens.
  * attention: head-parallel via AllToAll. Each core computes Q/K/V for its
    own tokens (all heads, feature-major, RoPE applied to Q/K), then one
    AllToAll redistributes K,V (and a second one Q) so core c holds heads
    {2c, 2c+1} for ALL 4096 (batch, position) tokens. Causal attention for
    those two heads runs fully on-chip (Q^T/K^T/V all SBUF-resident), and a
    third AllToAll routes the attention output back to token owners for the
    output projection. No all-reduce anywhere.

Layout: activations are feature-major (features on SBUF partitions, tokens on
the free axis) so every matmul is transpose-free: projections compute Y^T
directly (lhsT = weight column block, rhs = X^T), scores are built in S^T
orientation (kv on partitions) which feeds softmax(exp on ScalarE, key-padding
mask folded into the exp bias, causal diagonal handled by an additive band
mask) straight into the attention*V matmul. The softmax denominator is a
ones-column matmul accumulated alongside. RMSNorm statistics are computed with
a Square-activation + ones-matmul (cross-partition reduce on the PE).

Dtypes: float32r (full-rate fp32 PE mode) for all matmuls except ff2, which
runs in bf16 (h is cast at the silu activation so the 8192-wide hidden tensor
fits SBUF-resident and wf2 streams at half bandwidth).
"""
import sys

sys.path.insert(0, '/opt/trn_rl_repo')

import numpy as np
import ml_dtypes

import concourse.bacc as bacc
import concourse.mybir as mybir
from concourse import tile
from concourse.bass_utils import run_bass_kernel_spmd

F32 = mybir.dt.float32
F32R = mybir.dt.float32r
BF16 = mybir.dt.bfloat16
AF = mybir.ActivationFunctionType

D = 2048
H = 16
DH = 128
FF = 8192
B = 2
L = 2048
NCORES = 8
TOK = 512            # tokens per core
NF = D // 128        # 16 feature tiles
NEG = -30000.0
EPS = float(np.finfo(np.float32).eps)
ISQ = 1.0 / float(np.sqrt(DH))
RG = [list(range(NCORES))]


def _build():
    nc = bacc.Bacc("TRN2", target_bir_lowering=False, debug=False,
                   num_devices=NCORES)

    xT = nc.dram_tensor("xT", [D, TOK], F32, kind="ExternalInput")
    wq = nc.dram_tensor("wq", [D, D], F32R, kind="ExternalInput")
    wk = nc.dram_tensor("wk", [D, D], F32R, kind="ExternalInput")
    wv = nc.dram_tensor("wv", [D, D], F32R, kind="ExternalInput")
    wo = nc.dram_tensor("wo", [D, D], F32R, kind="ExternalInput")
    wf1 = nc.dram_tensor("wf1", [D, FF], F32R, kind="ExternalInput")
    wf2 = nc.dram_tensor("wf2", [FF, D], BF16, kind="ExternalInput")
    ropeC = nc.dram_tensor("ropeC", [DH, TOK], F32, kind="ExternalInput")
    ropeS2 = nc.dram_tensor("ropeS2", [DH, TOK], F32, kind="ExternalInput")
    band = nc.dram_tensor("band", [128, 896], F32, kind="ExternalInput")
    mbias = nc.dram_tensor("mbias", [128, 2 * H], F32, kind="ExternalInput")
    onesd = nc.dram_tensor("onesd", [128, 1], F32R, kind="ExternalInput")
    outT = nc.dram_tensor("outT", [D, TOK], F32, kind="ExternalOutput")

    # internal DRAM: AllToAll bounce buffers + spills
    kvin = nc.dram_tensor("kvin", [2 * D, TOK], F32R)
    kvout = nc.dram_tensor("kvout", [2 * D, TOK], F32R)
    qin = nc.dram_tensor("qin", [D, TOK], F32R)
    qout = nc.dram_tensor("qout", [D, TOK], F32R)
    oin = nc.dram_tensor("oin", [D, TOK], F32R)
    oout = nc.dram_tensor("oout", [D, TOK], F32R)
    x2d = nc.dram_tensor("x2d", [D, TOK], F32)

    with tile.TileContext(nc) as tc:
        with (
            tc.tile_pool(name="const", bufs=1) as cp,
            tc.tile_pool(name="small", bufs=1) as sp,
        ):
            cosT = cp.tile([DH, TOK], F32)
            sin2 = cp.tile([DH, TOK], F32)
            bandT = cp.tile([128, 896], F32)
            mbT = cp.tile([128, 2 * H], F32)
            onec = cp.tile([128, 1], F32R)
            epsc = cp.tile([1, 1], F32)
            nc.scalar.dma_start(cosT[:], ropeC[:])
            nc.scalar.dma_start(sin2[:], ropeS2[:])
            nc.scalar.dma_start(bandT[:], band[:])
            nc.scalar.dma_start(mbT[:], mbias[:])
            nc.scalar.dma_start(onec[:], onesd[:])
            nc.gpsimd.memset(epsc[:], EPS)

            rsB = sp.tile([128, TOK], F32)
            rowS = sp.tile([1, TOK], F32)
            rowR = sp.tile([1, TOK], F32)

            def rmsnorm_rs(ssq_ps):
                nc.scalar.activation(rowS[:], ssq_ps[:], AF.Sqrt,
                                     bias=epsc[:], scale=1.0 / D)
                nc.vector.reciprocal(rowR[:], rowS[:])
                nc.gpsimd.partition_broadcast(rsB[:], rowR[:])

            # ========== Phase 1: norm1, K^T, V, Q^T, AllToAlls ==========
            with (
                tc.tile_pool(name="m1", bufs=1) as m1,
                tc.tile_pool(name="ps1", bufs=3, space="PSUM") as ps1,
                tc.tile_pool(name="psr", bufs=1, space="PSUM") as psr,
            ):
                xt = m1.tile([128, NF * TOK], F32, tag="t1")
                nc.scalar.dma_start(
                    xt[:].rearrange("p (i c) -> p i c", i=NF),
                    xT[:].rearrange("(i p) c -> p i c", p=128))

                ssq = psr.tile([1, TOK], F32, tag="row")
                for i in range(NF):
                    sq = sp.tile([128, TOK], F32R, tag="sq", bufs=2)
                    nc.scalar.activation(sq[:], xt[:, i * TOK:(i + 1) * TOK],
                                         AF.Square)
                    nc.tensor.matmul(ssq[:], onec[:], sq[:],
                                     start=(i == 0), stop=(i == NF - 1))
                rmsnorm_rs(ssq)
                xnt = m1.tile([128, NF * TOK], F32R, tag="xn")
                for i in range(NF):
                    nc.vector.tensor_mul(xnt[:, i * TOK:(i + 1) * TOK],
                                         xt[:, i * TOK:(i + 1) * TOK], rsB[:])

                def proj_T(wten, out_tile, rope):
                    """out_tile[:, o*TOK:] = head-tile o of (xn @ w)^T."""
                    for o in range(NF):
                        wc = m1.tile([128, NF * 128], F32R, tag="wcol",
                                     bufs=3)
                        nc.sync.dma_start(
                            wc[:].rearrange("p (i m) -> p i m", i=NF),
                            wten[:, o * 128:(o + 1) * 128]
                            .rearrange("(i p) m -> p i m", p=128))
                        acc = ps1.tile([128, TOK], F32, tag="big")
                        for i in range(NF):
                            nc.tensor.matmul(
                                acc[:], wc[:, i * 128:(i + 1) * 128],
                                xnt[:, i * TOK:(i + 1) * TOK],
                                start=(i == 0), stop=(i == NF - 1))
                        dst = out_tile[:, o * TOK:(o + 1) * TOK]
                        if rope:
                            tmp = sp.tile([128, TOK], F32R, tag="rtmp",
                                          bufs=2)
                            nc.vector.tensor_mul(tmp[0:64, :], acc[64:128, :],
                                                 sin2[0:64, :])
                            nc.vector.tensor_mul(tmp[64:128, :], acc[0:64, :],
                                                 sin2[64:128, :])
                            nc.vector.tensor_mul(dst, acc[:], cosT[:])
                            nc.vector.tensor_add(dst, dst, tmp[:])
                        else:
                            nc.vector.tensor_copy(dst, acc[:])

                # K^T (roped) — reuses xt's slot (xt is dead after norm1)
                kt = m1.tile([128, NF * TOK], F32R, tag="t1")
                proj_T(wk, kt, rope=True)

                # V (token-major), 256-wide feature chunks
                vt = m1.tile([128, 4 * D], F32R, tag="t2")
                for fo in range(8):
                    wvc = m1.tile([128, NF * 256], F32R, tag="wv", bufs=2)
                    nc.scalar.dma_start(
                        wvc[:].rearrange("p (i m) -> p i m", i=NF),
                        wv[:, fo * 256:(fo + 1) * 256]
                        .rearrange("(i p) m -> p i m", p=128))
                    for to in range(4):
                        acc = ps1.tile([128, 256], F32, tag="big")
                        for i in range(NF):
                            nc.tensor.matmul(
                                acc[:],
                                xnt[:, i * TOK + to * 128:
                                    i * TOK + (to + 1) * 128],
                                wvc[:, i * 256:(i + 1) * 256],
                                start=(i == 0), stop=(i == NF - 1))
                        nc.vector.tensor_copy(
                            vt[:, to * D + fo * 256:to * D + (fo + 1) * 256],
                            acc[:])

                # bounce K+V bundle, kick AllToAll #1
                kv4 = kvin.ap().rearrange("(j q d) (t f) -> j d q t f",
                                          j=NCORES, q=4, d=128, t=2, f=256)
                vtv = vt[:].rearrange("p (t1 t2 j f) -> p t1 j t2 f",
                                      t1=2, t2=2, j=NCORES, f=256)
                for j in range(NCORES):
                    nc.gpsimd.dma_start(
                        kv4[j, :, 0:2, :, :]
                        .rearrange("d q t f -> d q (t f)"),
                        kt[:].rearrange("p (o c) -> p o c", o=NF)
                        [:, 2 * j:2 * j + 2, :])
                    for t1 in range(2):
                        nc.gpsimd.dma_start(
                            kv4[j, :, 2:4, t1, :],
                            vtv[:, t1, j, :, :])
                nc.gpsimd.collective_compute(
                    "AllToAll", mybir.AluOpType.bypass, replica_groups=RG,
                    ins=[kvin.ap().opt()], outs=[kvout.ap().opt()])

                # Q^T (roped), bounce, AllToAll #2 — reuses vt's slot
                qt = m1.tile([128, NF * TOK], F32R, tag="t2")
                proj_T(wq, qt, rope=True)
                qiv = qin.ap().rearrange("(j s d) c -> j d s c", j=NCORES,
                                         s=2, d=128)
                for j in range(NCORES):
                    nc.gpsimd.dma_start(
                        qiv[j], qt[:].rearrange("p (o c) -> p o c", o=NF)
                        [:, 2 * j:2 * j + 2, :])
                nc.gpsimd.collective_compute(
                    "AllToAll", mybir.AluOpType.bypass, replica_groups=RG,
                    ins=[qin.ap().opt()], outs=[qout.ap().opt()])

            # ============ Phase 2: attention (heads 2c, 2c+1) ============
            with (
                tc.tile_pool(name="m2", bufs=1) as m2,
                tc.tile_pool(name="ps_s", bufs=2, space="PSUM") as ps_s,
                tc.tile_pool(name="ps_av", bufs=2, space="PSUM") as ps_av,
                tc.tile_pool(name="ps_dn", bufs=2, space="PSUM") as ps_dn,
            ):
                ksb = m2.tile([128, 2 * 4096], F32R)   # [dh, hh, (b,pos)]
                vsb = m2.tile([128, 32 * 256], F32R)   # [kv%128, tile, feat]
                qsb = m2.tile([128, 2 * 4096], F32R)
                osb = m2.tile([128, 2 * 4096], F32R)
                kv4o = kvout.ap().rearrange("(j q d) (t f) -> j d q t f",
                                            j=NCORES, q=4, d=128, t=2, f=256)
                ksbv = ksb[:].rearrange("d (h j c) -> d h j c", h=2, j=NCORES)
                vsbv = vsb[:].rearrange(
                    "p (jj t1 t2 f) -> p jj t1 t2 f", jj=NCORES, t1=2, t2=2,
                    f=256)
                qsbv = qsb[:].rearrange("d (h j c) -> d h j c", h=2, j=NCORES)
                qov = qout.ap().rearrange("(j s d) c -> j d s c", j=NCORES,
                                          s=2, d=128)
                for j in range(NCORES):
                    nc.scalar.dma_start(
                        ksbv[:, :, j, :],
                        kv4o[j, :, 0:2, :, :]
                        .rearrange("d q t f -> d q (t f)"))
                    for t1 in range(2):
                        nc.scalar.dma_start(
                            vsbv[:, j, t1, :, :], kv4o[j, :, 2:4, t1, :])
                    nc.scalar.dma_start(qsbv[:, :, j, :], qov[j])

                for b in range(B):
                    for hh in range(2):
                        for q4 in range(4):
                            qs = qsb[:, hh * 4096 + b * 2048 + q4 * 512:
                                     hh * 4096 + b * 2048 + (q4 + 1) * 512]
                            ng = 4 * q4 + 4
                            av = ps_av.tile([128, 512], F32, tag="av")
                            dn = ps_dn.tile([1, 512], F32, tag="dn")
                            for g in range(ng):
                                st = ps_s.tile([128, 512], F32, tag="s")
                                nc.tensor.matmul(
                                    st[:],
                                    ksb[:, hh * 4096 + b * 2048 + g * 128:
                                        hh * 4096 + b * 2048 + (g + 1) * 128],
                                    qs, start=True, stop=True)
                                if g >= 4 * q4:
                                    r = (g - 4 * q4) * 128
                                    nc.vector.tensor_add(
                                        st[:], st[:],
                                        bandT[:, 384 - r:896 - r])
                                pt = sp.tile([128, 512], F32R, tag="pt",
                                             bufs=3)
                                nc.scalar.activation(
                                    pt[:], st[:], AF.Exp,
                                    bias=mbT[:, b * H + g:b * H + g + 1],
                                    scale=ISQ)
                                nc.tensor.matmul(dn[:], onec[:], pt[:],
                                                 start=(g == 0),
                                                 stop=(g == ng - 1))
                                nc.tensor.matmul(
                                    av[:],
                                    vsb[:, (b * H + g) * 256 + hh * 128:
                                        (b * H + g) * 256 + (hh + 1) * 128],
                                    pt[:], start=(g == 0), stop=(g == ng - 1))
                            dnr = sp.tile([1, 512], F32, tag="dnr", bufs=2)
                            nc.vector.reciprocal(dnr[:], dn[:])
                            rdB = sp.tile([128, 512], F32, tag="rdB", bufs=2)
                            nc.gpsimd.partition_broadcast(rdB[:], dnr[:])
                            nc.vector.tensor_mul(
                                osb[:, hh * 4096 + b * 2048 + q4 * 512:
                                    hh * 4096 + b * 2048 + (q4 + 1) * 512],
                                av[:], rdB[:])

                oiv = oin.ap().rearrange("(j s d) c -> j d s c", j=NCORES,
                                         s=2, d=128)
                osv = osb[:].rearrange("d (h j c) -> d h j c", h=2, j=NCORES)
                for j in range(NCORES):
                    nc.gpsimd.dma_start(oiv[j], osv[:, :, j, :])
                nc.gpsimd.collective_compute(
                    "AllToAll", mybir.AluOpType.bypass, replica_groups=RG,
                    ins=[oin.ap().opt()], outs=[oout.ap().opt()])

            # ======= Phase 3: O-projection + residual + norm2 stats ======
            with (
                tc.tile_pool(name="m3", bufs=1) as m3,
                tc.tile_pool(name="ps3", bufs=2, space="PSUM") as ps3,
                tc.tile_pool(name="psr3", bufs=1, space="PSUM") as psr3,
            ):
                ao = m3.tile([128, NF * TOK], F32R)  # attnT, all heads
                nc.scalar.dma_start(
                    ao[:].rearrange("p (i c) -> p i c", i=NF),
                    oout.ap().rearrange("(i p) c -> p i c", p=128))
                ssq2 = psr3.tile([1, TOK], F32, tag="row")
                for o in range(NF):
                    wc = m3.tile([128, NF * 128], F32R, tag="wocol", bufs=3)
                    nc.sync.dma_start(
                        wc[:].rearrange("p (i m) -> p i m", i=NF),
                        wo[:, o * 128:(o + 1) * 128]
                        .rearrange("(i p) m -> p i m", p=128))
                    acc = ps3.tile([128, TOK], F32, tag="big")
                    for i in range(NF):
                        nc.tensor.matmul(acc[:], wc[:, i * 128:(i + 1) * 128],
                                         ao[:, i * TOK:(i + 1) * TOK],
                                         start=(i == 0), stop=(i == NF - 1))
                    xsl = m3.tile([128, TOK], F32, tag="xsl", bufs=2)
                    nc.scalar.dma_start(xsl[:], xT[o * 128:(o + 1) * 128, :])
                    x2sl = m3.tile([128, TOK], F32, tag="x2sl", bufs=2)
                    nc.vector.tensor_add(x2sl[:], xsl[:], acc[:])
                    nc.scalar.dma_start(x2d[o * 128:(o + 1) * 128, :],
                                        x2sl[:])
                    sq = sp.tile([128, TOK], F32R, tag="sq", bufs=2)
                    nc.scalar.activation(sq[:], x2sl[:], AF.Square)
                    nc.tensor.matmul(ssq2[:], onec[:], sq[:],
                                     start=(o == 0), stop=(o == NF - 1))
                rmsnorm_rs(ssq2)

            # ==================== Phase 4: norm2 + FFN ===================
            with (
                tc.tile_pool(name="m4", bufs=1) as m4,
                tc.tile_pool(name="ps4", bufs=3, space="PSUM") as ps4,
            ):
                xn2 = m4.tile([128, NF * TOK], F32R)
                for i in range(NF):
                    xsl = m4.tile([128, TOK], F32, tag="xsl", bufs=2)
                    nc.scalar.dma_start(xsl[:], x2d[i * 128:(i + 1) * 128, :])
                    nc.vector.tensor_mul(xn2[:, i * TOK:(i + 1) * TOK],
                                         xsl[:], rsB[:])
                # ff1 + silu -> h (bf16, SBUF resident)
                h = m4.tile([128, 64 * TOK], BF16)
                for o in range(FF // 128):
                    wc = m4.tile([128, NF * 128], F32R, tag="wf1c", bufs=3)
                    nc.sync.dma_start(
                        wc[:].rearrange("p (i m) -> p i m", i=NF),
                        wf1[:, o * 128:(o + 1) * 128]
                        .rearrange("(i p) m -> p i m", p=128))
                    acc = ps4.tile([128, TOK], F32, tag="big")
                    for i in range(NF):
                        nc.tensor.matmul(acc[:], wc[:, i * 128:(i + 1) * 128],
                                         xn2[:, i * TOK:(i + 1) * TOK],
                                         start=(i == 0), stop=(i == NF - 1))
                    nc.scalar.activation(h[:, o * TOK:(o + 1) * TOK], acc[:],
                                         AF.Silu)
                # ff2 (bf16) + residual -> outT
                for o in range(NF):
                    wc2 = m4.tile([128, 64 * 128], BF16, tag="wf2c", bufs=2)
                    nc.scalar.dma_start(
                        wc2[:].rearrange("p (k m) -> p k m", k=64),
                        wf2[:, o * 128:(o + 1) * 128]
                        .rearrange("(k p) m -> p k m", p=128))
                    acc = ps4.tile([128, TOK], F32, tag="big")
                    for k in range(64):
                        nc.tensor.matmul(acc[:], wc2[:, k * 128:(k + 1) * 128],
                                         h[:, k * TOK:(k + 1) * TOK],
                                         start=(k == 0), stop=(k == 63))
                    xsl = m4.tile([128, TOK], F32, tag="xsl", bufs=2)
                    nc.scalar.dma_start(xsl[:], x2d[o * 128:(o + 1) * 128, :])
                    osl = m4.tile([128, TOK], F32, tag="osl", bufs=2)
                    nc.vector.tensor_add(osl[:], xsl[:], acc[:])
                    nc.sync.dma_start(outT[o * 128:(o + 1) * 128, :], osl[:])

    nc.compile()
    return nc


_COMPILED = None


def _prep_inmaps(x, rope_cos, rope_sin, mask, w_norm1, w_norm2, wq, wk, wv,
                 wo, w_ff1, w_ff2):
    x = np.asarray(x, np.float32)
    cos = np.asarray(rope_cos, np.float32)
    sin = np.asarray(rope_sin, np.float32)
    mask = np.asarray(mask)
    wn1 = np.asarray(w_norm1, np.float32)
    wn2 = np.asarray(w_norm2, np.float32)

    wqn = np.ascontiguousarray(wn1[:, None] * np.asarray(wq, np.float32))
    wkn = np.ascontiguousarray(wn1[:, None] * np.asarray(wk, np.float32))
    wvn = np.ascontiguousarray(wn1[:, None] * np.asarray(wv, np.float32))
    won = np.ascontiguousarray(np.asarray(wo, np.float32))
    wf1n = np.ascontiguousarray(wn2[:, None] * np.asarray(w_ff1, np.float32))
    wf2b = np.asarray(w_ff2, np.float32).astype(ml_dtypes.bfloat16)

    # causal band mask: band[row, cc] = 0 iff cc >= row + 384
    cc = np.arange(896)[None, :]
    rr = np.arange(128)[:, None]
    band = np.where(cc >= rr + 384, 0.0, NEG).astype(np.float32)
    # key-padding mask bias, [128, 2*H]: col b*16+g <- kv pos 128g+p
    mb = np.where(mask != 0, 0.0, NEG).astype(np.float32)  # [B, L]
    mbias = np.ascontiguousarray(
        mb.reshape(B, H, 128).transpose(2, 0, 1).reshape(128, B * H))

    in_maps = []
    for c in range(NCORES):
        b = c // 4
        lo = 512 * (c % 4)
        pos = slice(lo, lo + TOK)
        s = sin[pos].T.copy()
        s2 = np.concatenate([-s[:64], s[64:]], axis=0)
        in_maps.append({
            "xT": np.ascontiguousarray(x[b, pos].T),
            "wq": wqn, "wk": wkn, "wv": wvn, "wo": won,
            "wf1": wf1n, "wf2": wf2b,
            "ropeC": np.ascontiguousarray(cos[pos].T),
            "ropeS2": np.ascontiguousarray(s2),
            "band": band, "mbias": mbias,
            "onesd": np.ones((128, 1), np.float32),
        })
    return in_maps


def _assemble(res):
    out = np.empty((B, L, D), np.float32)
    for c in range(NCORES):
        b = c // 4
        lo = 512 * (c % 4)
        out[b, lo:lo + TOK, :] = res.results[c]["outT"].T
    return out


def kernel(**inputs):
    global _COMPILED
    if _COMPILED is None:
        _COMPILED = _build()
    in_maps = _prep_inmaps(**inputs)
    res = run_bass_kernel_spmd(_COMPILED, in_maps, list(range(NCORES)))
    return _assemble(res)


def timed_run(**inputs):
    """Run with NTFF profiling; returns (exec_time_ns, BassKernelResults)."""
    global _COMPILED
    if _COMPILED is None:
        _COMPILED = _build()
    in_maps = _prep_inmaps(**inputs)
    res = run_bass_kernel_spmd(_COMPILED, in_maps, list(range(NCORES)),
                               trace=True)
    return res.exec_time_ns, res



# revision 6
# speedup vs baseline: 1.3573x; 1.3573x over previous
"""Trainium2 Bass kernel for a dense transformer decoder block on 8 NeuronCores.

Sharding (uniform SPMD):
  * tokens: core c owns 512 contiguous tokens — batch c//4, positions
    [512*(c%4), 512*(c%4)+512). All projections, norms and the FFN are
    computed purely locally on those tokens.
  * attention: head-parallel via AllToAll. Each core computes Q/K/V for its
    own tokens (all heads, feature-major, RoPE applied to Q/K), then one
    AllToAll redistributes K,V (and a second one Q) so core c holds heads
    {2c, 2c+1} for ALL 4096 (batch, position) tokens. Causal attention for
    those two heads runs fully on-chip, and a third AllToAll routes the
    attention output back to token owners for the output projection.

Performance notes (v2):
  * all matmuls run in bf16 (fp32 PSUM accumulation) — fp32 matmuls stream
    at ~2 cycles/row on TRN2, bf16 at 1; weight DMA traffic also halves.
  * weights are pre-tiled host-side into [tile, 128, cols] layout so every
    weight DMA is a single long contiguous run per partition (4-16KB
    descriptors instead of 512B).
  * the attention residual x2 stays SBUF-resident between phases 3 and 4.
  * AllToAll payloads are bf16.
"""
import sys

sys.path.insert(0, '/opt/trn_rl_repo')

import numpy as np
import ml_dtypes

import concourse.bacc as bacc
import concourse.mybir as mybir
from concourse import tile
from concourse.bass_utils import run_bass_kernel_spmd

F32 = mybir.dt.float32
BF16 = mybir.dt.bfloat16
AF = mybir.ActivationFunctionType

D = 2048
H = 16
DH = 128
FF = 8192
B = 2
L = 2048
NCORES = 8
TOK = 512            # tokens per core
NF = D // 128        # 16 feature tiles
NEG = -30000.0
EPS = float(np.finfo(np.float32).eps)
ISQ = 1.0 / float(np.sqrt(DH))
RG = [list(range(NCORES))]


def _build():
    nc = bacc.Bacc("TRN2", target_bir_lowering=False, debug=False,
                   num_devices=NCORES)

    xT = nc.dram_tensor("xT", [D, TOK], F32, kind="ExternalInput")
    # pre-tiled weights: [out_tile, 128, in_tile*tile_w] bf16, contiguous
    wq = nc.dram_tensor("wq", [NF, 128, NF * 128], BF16, kind="ExternalInput")
    wk = nc.dram_tensor("wk", [NF, 128, NF * 128], BF16, kind="ExternalInput")
    wv = nc.dram_tensor("wv", [4, 128, NF * 512], BF16, kind="ExternalInput")
    wo = nc.dram_tensor("wo", [NF, 128, NF * 128], BF16, kind="ExternalInput")
    wf1 = nc.dram_tensor("wf1", [FF // 128, 128, NF * 128], BF16,
                         kind="ExternalInput")
    wf2 = nc.dram_tensor("wf2", [NF, 128, 64 * 128], BF16,
                         kind="ExternalInput")
    ropeC = nc.dram_tensor("ropeC", [DH, TOK], F32, kind="ExternalInput")
    ropeS2 = nc.dram_tensor("ropeS2", [DH, TOK], F32, kind="ExternalInput")
    band = nc.dram_tensor("band", [128, 896], F32, kind="ExternalInput")
    mbias = nc.dram_tensor("mbias", [128, 2 * H], F32, kind="ExternalInput")
    onesd = nc.dram_tensor("onesd", [128, 1], BF16, kind="ExternalInput")
    outT = nc.dram_tensor("outT", [D, TOK], F32, kind="ExternalOutput")

    # internal DRAM: AllToAll bounce buffers (bf16)
    kvin = nc.dram_tensor("kvin", [2 * D, TOK], BF16)
    kvout = nc.dram_tensor("kvout", [2 * D, TOK], BF16)
    qin = nc.dram_tensor("qin", [D, TOK], BF16)
    qout = nc.dram_tensor("qout", [D, TOK], BF16)
    oin = nc.dram_tensor("oin", [D, TOK], BF16)
    oout = nc.dram_tensor("oout", [D, TOK], BF16)

    with tile.TileContext(nc) as tc:
        with (
            tc.tile_pool(name="const", bufs=1) as cp,
            tc.tile_pool(name="small", bufs=1) as sp,
        ):
            cosT = cp.tile([DH, TOK], F32)
            sin2 = cp.tile([DH, TOK], F32)
            bandT = cp.tile([128, 896], F32)
            mbT = cp.tile([128, 2 * H], F32)
            onec = cp.tile([128, 1], BF16)
            epsc = cp.tile([1, 1], F32)
            nc.scalar.dma_start(cosT[:], ropeC[:])
            nc.scalar.dma_start(sin2[:], ropeS2[:])
            nc.scalar.dma_start(bandT[:], band[:])
            nc.scalar.dma_start(mbT[:], mbias[:])
            nc.scalar.dma_start(onec[:], onesd[:])
            nc.gpsimd.memset(epsc[:], EPS)

            rsB = sp.tile([128, TOK], F32)
            rowS = sp.tile([1, TOK], F32)
            rowR = sp.tile([1, TOK], F32)

            def rmsnorm_rs(ssq_ps):
                nc.scalar.activation(rowS[:], ssq_ps[:], AF.Sqrt,
                                     bias=epsc[:], scale=1.0 / D)
                nc.vector.reciprocal(rowR[:], rowS[:])
                nc.gpsimd.partition_broadcast(rsB[:], rowR[:])

            # ========== Phase 1: norm1, K^T, V, Q^T, AllToAlls ==========
            with (
                tc.tile_pool(name="m1", bufs=1) as m1,
                tc.tile_pool(name="ps1", bufs=3, space="PSUM") as ps1,
                tc.tile_pool(name="psr", bufs=1, space="PSUM") as psr,
            ):
                # load x in 4 column groups so norm stats start early
                xt = m1.tile([128, NF * TOK], F32, tag="t1")
                xtv = xt[:].rearrange("p (i c) -> p i c", i=NF)
                xTv = xT[:].rearrange("(i p) c -> p i c", p=128)
                for ig in range(4):
                    nc.scalar.dma_start(xtv[:, 4 * ig:4 * ig + 4, :],
                                        xTv[:, 4 * ig:4 * ig + 4, :])

                ssq = psr.tile([1, TOK], F32, tag="row")
                for i in range(NF):
                    sq = sp.tile([128, TOK], BF16, tag="sq", bufs=2)
                    nc.scalar.activation(sq[:], xt[:, i * TOK:(i + 1) * TOK],
                                         AF.Square)
                    nc.tensor.matmul(ssq[:], onec[:], sq[:],
                                     start=(i == 0), stop=(i == NF - 1))
                rmsnorm_rs(ssq)
                xnt = m1.tile([128, NF * TOK], BF16, tag="xn")
                for i in range(NF):
                    nc.vector.tensor_mul(xnt[:, i * TOK:(i + 1) * TOK],
                                         xt[:, i * TOK:(i + 1) * TOK], rsB[:])

                def proj_T(wten, out_tile, rope):
                    """out_tile[:, o*TOK:] = head-tile o of (xn @ w)^T, bf16."""
                    for o in range(NF):
                        wc = m1.tile([128, NF * 128], BF16, tag="wcol",
                                     bufs=3)
                        nc.sync.dma_start(wc[:], wten.ap()[o])
                        acc = ps1.tile([128, TOK], F32, tag="big")
                        for i in range(NF):
                            nc.tensor.matmul(
                                acc[:], wc[:, i * 128:(i + 1) * 128],
                                xnt[:, i * TOK:(i + 1) * TOK],
                                start=(i == 0), stop=(i == NF - 1))
                        dst = out_tile[:, o * TOK:(o + 1) * TOK]
                        if rope:
                            tmp = sp.tile([128, TOK], F32, tag="rtmp",
                                          bufs=2)
                            tmp2 = sp.tile([128, TOK], F32, tag="rtmp2",
                                           bufs=2)
                            nc.vector.tensor_mul(tmp[0:64, :], acc[64:128, :],
                                                 sin2[0:64, :])
                            nc.vector.tensor_mul(tmp[64:128, :], acc[0:64, :],
                                                 sin2[64:128, :])
                            nc.vector.tensor_mul(tmp2[:], acc[:], cosT[:])
                            nc.vector.tensor_add(dst, tmp2[:], tmp[:])
                        else:
                            nc.vector.tensor_copy(dst, acc[:])

                # K^T (roped) — reuses xt's slot (xt dead after xnt)
                kt = m1.tile([128, NF * TOK], BF16, tag="t1")
                proj_T(wk, kt, rope=True)

                # V (token-major), 512-wide feature chunks
                vt = m1.tile([128, 4 * D], BF16, tag="t2")
                for fo in range(4):
                    wvc = m1.tile([128, NF * 512], BF16, tag="wv", bufs=2)
                    nc.sync.dma_start(wvc[:], wv.ap()[fo])
                    for to in range(4):
                        acc = ps1.tile([128, TOK], F32, tag="big")
                        for i in range(NF):
                            nc.tensor.matmul(
                                acc[:],
                                xnt[:, i * TOK + to * 128:
                                    i * TOK + (to + 1) * 128],
                                wvc[:, i * 512:(i + 1) * 512],
                                start=(i == 0), stop=(i == NF - 1))
                        nc.vector.tensor_copy(
                            vt[:, to * D + fo * 512:to * D + (fo + 1) * 512],
                            acc[:])

                # bounce K+V bundle, kick AllToAll #1
                kv4 = kvin.ap().rearrange("(j q d) (t f) -> j d q t f",
                                          j=NCORES, q=4, d=128, t=2, f=256)
                vtv = vt[:].rearrange("p (t1 t2 j f) -> p t1 j t2 f",
                                      t1=2, t2=2, j=NCORES, f=256)
                for j in range(NCORES):
                    nc.gpsimd.dma_start(
                        kv4[j, :, 0:2, :, :]
                        .rearrange("d q t f -> d q (t f)"),
                        kt[:].rearrange("p (o c) -> p o c", o=NF)
                        [:, 2 * j:2 * j + 2, :])
                    for t1 in range(2):
                        nc.gpsimd.dma_start(
                            kv4[j, :, 2:4, t1, :],
                            vtv[:, t1, j, :, :])
                nc.gpsimd.collective_compute(
                    "AllToAll", mybir.AluOpType.bypass, replica_groups=RG,
                    ins=[kvin.ap().opt()], outs=[kvout.ap().opt()])

                # Q^T (roped), bounce, AllToAll #2 — reuses vt's slot
                qt = m1.tile([128, NF * TOK], BF16, tag="t2")
                proj_T(wq, qt, rope=True)
                qiv = qin.ap().rearrange("(j s d) c -> j d s c", j=NCORES,
                                         s=2, d=128)
                for j in range(NCORES):
                    nc.gpsimd.dma_start(
                        qiv[j], qt[:].rearrange("p (o c) -> p o c", o=NF)
                        [:, 2 * j:2 * j + 2, :])
                nc.gpsimd.collective_compute(
                    "AllToAll", mybir.AluOpType.bypass, replica_groups=RG,
                    ins=[qin.ap().opt()], outs=[qout.ap().opt()])

            # ============ Phase 2: attention (heads 2c, 2c+1) ============
            with (
                tc.tile_pool(name="m2", bufs=1) as m2,
                tc.tile_pool(name="ps_s", bufs=3, space="PSUM") as ps_s,
                tc.tile_pool(name="ps_av", bufs=2, space="PSUM") as ps_av,
                tc.tile_pool(name="ps_dn", bufs=2, space="PSUM") as ps_dn,
            ):
                ksb = m2.tile([128, 2 * 4096], BF16)   # [dh, hh, (b,pos)]
                vsb = m2.tile([128, 32 * 256], BF16)   # [kv%128, tile, feat]
                qsb = m2.tile([128, 2 * 4096], BF16)
                osb = m2.tile([128, 2 * 4096], BF16)
                kv4o = kvout.ap().rearrange("(j q d) (t f) -> j d q t f",
                                            j=NCORES, q=4, d=128, t=2, f=256)
                ksbv = ksb[:].rearrange("d (h j c) -> d h j c", h=2, j=NCORES)
                vsbv = vsb[:].rearrange(
                    "p (jj t1 t2 f) -> p jj t1 t2 f", jj=NCORES, t1=2, t2=2,
                    f=256)
                qsbv = qsb[:].rearrange("d (h j c) -> d h j c", h=2, j=NCORES)
                qov = qout.ap().rearrange("(j s d) c -> j d s c", j=NCORES,
                                          s=2, d=128)
                for j in range(NCORES):
                    nc.scalar.dma_start(
                        ksbv[:, :, j, :],
                        kv4o[j, :, 0:2, :, :]
                        .rearrange("d q t f -> d q (t f)"))
                    for t1 in range(2):
                        nc.scalar.dma_start(
                            vsbv[:, j, t1, :, :], kv4o[j, :, 2:4, t1, :])
                    nc.scalar.dma_start(qsbv[:, :, j, :], qov[j])

                for b in range(B):
                    for hh in range(2):
                        for q4 in range(4):
                            qs = qsb[:, hh * 4096 + b * 2048 + q4 * 512:
                                     hh * 4096 + b * 2048 + (q4 + 1) * 512]
                            ng = 4 * q4 + 4
                            av = ps_av.tile([128, 512], F32, tag="av")
                            dn = ps_dn.tile([1, 512], F32, tag="dn")
                            for g in range(ng):
                                st = ps_s.tile([128, 512], F32, tag="s")
                                nc.tensor.matmul(
                                    st[:],
                                    ksb[:, hh * 4096 + b * 2048 + g * 128:
                                        hh * 4096 + b * 2048 + (g + 1) * 128],
                                    qs, start=True, stop=True)
                                if g >= 4 * q4:
                                    r = (g - 4 * q4) * 128
                                    nc.vector.tensor_add(
                                        st[:], st[:],
                                        bandT[:, 384 - r:896 - r])
                                pt = sp.tile([128, 512], BF16, tag="pt",
                                             bufs=3)
                                nc.scalar.activation(
                                    pt[:], st[:], AF.Exp,
                                    bias=mbT[:, b * H + g:b * H + g + 1],
                                    scale=ISQ)
                                nc.tensor.matmul(dn[:], onec[:], pt[:],
                                                 start=(g == 0),
                                                 stop=(g == ng - 1))
                                nc.tensor.matmul(
                                    av[:],
                                    vsb[:, (b * H + g) * 256 + hh * 128:
                                        (b * H + g) * 256 + (hh + 1) * 128],
                                    pt[:], start=(g == 0), stop=(g == ng - 1))
                            dnr = sp.tile([1, 512], F32, tag="dnr", bufs=2)
                            nc.vector.reciprocal(dnr[:], dn[:])
                            rdB = sp.tile([128, 512], F32, tag="rdB", bufs=2)
                            nc.gpsimd.partition_broadcast(rdB[:], dnr[:])
                            nc.vector.tensor_mul(
                                osb[:, hh * 4096 + b * 2048 + q4 * 512:
                                    hh * 4096 + b * 2048 + (q4 + 1) * 512],
                                av[:], rdB[:])

                oiv = oin.ap().rearrange("(j s d) c -> j d s c", j=NCORES,
                                         s=2, d=128)
                osv = osb[:].rearrange("d (h j c) -> d h j c", h=2, j=NCORES)
                for j in range(NCORES):
                    nc.gpsimd.dma_start(oiv[j], osv[:, :, j, :])
                nc.gpsimd.collective_compute(
                    "AllToAll", mybir.AluOpType.bypass, replica_groups=RG,
                    ins=[oin.ap().opt()], outs=[oout.ap().opt()])

            # x2 (attention residual) stays SBUF-resident through phase 4
            with tc.tile_pool(name="x2p", bufs=1) as x2p:
                x2sb = x2p.tile([128, NF * TOK], F32)

                # ===== Phase 3: O-projection + residual + norm2 stats =====
                with (
                    tc.tile_pool(name="m3", bufs=1) as m3,
                    tc.tile_pool(name="ps3", bufs=2, space="PSUM") as ps3,
                    tc.tile_pool(name="psr3", bufs=1, space="PSUM") as psr3,
                ):
                    ao = m3.tile([128, NF * TOK], BF16)  # attnT, all heads
                    aov = ao[:].rearrange("p (i c) -> p i c", i=NF)
                    oov = oout.ap().rearrange("(i p) c -> p i c", p=128)
                    for ig in range(4):
                        nc.scalar.dma_start(aov[:, 4 * ig:4 * ig + 4, :],
                                            oov[:, 4 * ig:4 * ig + 4, :])
                    ssq2 = psr3.tile([1, TOK], F32, tag="row")
                    for o in range(NF):
                        wc = m3.tile([128, NF * 128], BF16, tag="wocol",
                                     bufs=3)
                        nc.sync.dma_start(wc[:], wo.ap()[o])
                        acc = ps3.tile([128, TOK], F32, tag="big")
                        for i in range(NF):
                            nc.tensor.matmul(
                                acc[:], wc[:, i * 128:(i + 1) * 128],
                                ao[:, i * TOK:(i + 1) * TOK],
                                start=(i == 0), stop=(i == NF - 1))
                        xsl = m3.tile([128, TOK], F32, tag="xsl", bufs=2)
                        nc.scalar.dma_start(xsl[:],
                                            xT[o * 128:(o + 1) * 128, :])
                        x2sl = x2sb[:, o * TOK:(o + 1) * TOK]
                        nc.vector.tensor_add(x2sl, xsl[:], acc[:])
                        sq = sp.tile([128, TOK], BF16, tag="sq", bufs=2)
                        nc.scalar.activation(sq[:], x2sl, AF.Square)
                        nc.tensor.matmul(ssq2[:], onec[:], sq[:],
                                         start=(o == 0), stop=(o == NF - 1))
                    rmsnorm_rs(ssq2)

                # ================== Phase 4: norm2 + FFN ==================
                with (
                    tc.tile_pool(name="m4", bufs=1) as m4,
                    tc.tile_pool(name="ps4", bufs=3, space="PSUM") as ps4,
                ):
                    xn2 = m4.tile([128, NF * TOK], BF16)
                    for i in range(NF):
                        nc.vector.tensor_mul(
                            xn2[:, i * TOK:(i + 1) * TOK],
                            x2sb[:, i * TOK:(i + 1) * TOK], rsB[:])
                    # ff1 + silu -> h (bf16, SBUF resident)
                    h = m4.tile([128, 64 * TOK], BF16)
                    for o in range(FF // 128):
                        wc = m4.tile([128, NF * 128], BF16, tag="wf1c",
                                     bufs=3)
                        nc.sync.dma_start(wc[:], wf1.ap()[o])
                        acc = ps4.tile([128, TOK], F32, tag="big")
                        for i in range(NF):
                            nc.tensor.matmul(
                                acc[:], wc[:, i * 128:(i + 1) * 128],
                                xn2[:, i * TOK:(i + 1) * TOK],
                                start=(i == 0), stop=(i == NF - 1))
                        nc.scalar.activation(h[:, o * TOK:(o + 1) * TOK],
                                             acc[:], AF.Silu)
                    # ff2 (bf16) + residual -> outT
                    for o in range(NF):
                        wc2 = m4.tile([128, 64 * 128], BF16, tag="wf2c",
                                      bufs=2)
                        nc.sync.dma_start(wc2[:], wf2.ap()[o])
                        acc = ps4.tile([128, TOK], F32, tag="big")
                        for k in range(64):
                            nc.tensor.matmul(
                                acc[:], wc2[:, k * 128:(k + 1) * 128],
                                h[:, k * TOK:(k + 1) * TOK],
                                start=(k == 0), stop=(k == 63))
                        osl = m4.tile([128, TOK], F32, tag="osl", bufs=2)
                        nc.vector.tensor_add(
                            osl[:], x2sb[:, o * TOK:(o + 1) * TOK], acc[:])
                        nc.sync.dma_start(outT[o * 128:(o + 1) * 128, :],
                                          osl[:])

    nc.compile()
    return nc


_COMPILED = None


def _tile_oi(w, tile_w):
    """[D_in, n_out*tile_w] -> [n_out, 128, (D_in//128)*tile_w] bf16."""
    d_in, d_out = w.shape
    n_i, n_o = d_in // 128, d_out // tile_w
    t = w.reshape(n_i, 128, n_o, tile_w).transpose(2, 1, 0, 3)
    return np.ascontiguousarray(t.reshape(n_o, 128, n_i * tile_w)
                                .astype(ml_dtypes.bfloat16))


def _prep_inmaps(x, rope_cos, rope_sin, mask, w_norm1, w_norm2, wq, wk, wv,
                 wo, w_ff1, w_ff2):
    x = np.asarray(x, np.float32)
    cos = np.asarray(rope_cos, np.float32)
    sin = np.asarray(rope_sin, np.float32)
    mask = np.asarray(mask)
    wn1 = np.asarray(w_norm1, np.float32)
    wn2 = np.asarray(w_norm2, np.float32)

    wqt = _tile_oi(wn1[:, None] * np.asarray(wq, np.float32), 128)
    wkt = _tile_oi(wn1[:, None] * np.asarray(wk, np.float32), 128)
    wvt = _tile_oi(wn1[:, None] * np.asarray(wv, np.float32), 512)
    wot = _tile_oi(np.asarray(wo, np.float32), 128)
    wf1t = _tile_oi(wn2[:, None] * np.asarray(w_ff1, np.float32), 128)
    wf2t = _tile_oi(np.asarray(w_ff2, np.float32), 128)

    # causal band mask: band[row, cc] = 0 iff cc >= row + 384
    cc = np.arange(896)[None, :]
    rr = np.arange(128)[:, None]
    band = np.where(cc >= rr + 384, 0.0, NEG).astype(np.float32)
    # key-padding mask bias, [128, 2*H]: col b*16+g <- kv pos 128g+p
    mb = np.where(mask != 0, 0.0, NEG).astype(np.float32)  # [B, L]
    mbias = np.ascontiguousarray(
        mb.reshape(B, H, 128).transpose(2, 0, 1).reshape(128, B * H))

    in_maps = []
    for c in range(NCORES):
        b = c // 4
        lo = 512 * (c % 4)
        pos = slice(lo, lo + TOK)
        s = sin[pos].T.copy()
        s2 = np.concatenate([-s[:64], s[64:]], axis=0)
        in_maps.append({
            "xT": np.ascontiguousarray(x[b, pos].T),
            "wq": wqt, "wk": wkt, "wv": wvt, "wo": wot,
            "wf1": wf1t, "wf2": wf2t,
            "ropeC": np.ascontiguousarray(cos[pos].T),
            "ropeS2": np.ascontiguousarray(s2),
            "band": band, "mbias": mbias,
            "onesd": np.ones((128, 1), ml_dtypes.bfloat16),
        })
    return in_maps


def _assemble(res):
    out = np.empty((B, L, D), np.float32)
    for c in range(NCORES):
        b = c // 4
        lo = 512 * (c % 4)
        out[b, lo:lo + TOK, :] = res.results[c]["outT"].T
    return out


def kernel(**inputs):
    global _COMPILED
    if _COMPILED is None:
        _COMPILED = _build()
    in_maps = _prep_inmaps(**inputs)
    res = run_bass_kernel_spmd(_COMPILED, in_maps, list(range(NCORES)))
    return _assemble(res)


def timed_run(**inputs):
    """Run with NTFF profiling; returns (exec_time_ns, BassKernelResults)."""
    global _COMPILED
    if _COMPILED is None:
        _COMPILED = _build()
    in_maps = _prep_inmaps(**inputs)
    res = run_bass_kernel_spmd(_COMPILED, in_maps, list(range(NCORES)),
                               trace=True)
    return res.exec_time_ns, res
